# revision 17
# baseline (speedup 1.0000x reference)
"""Multi-head attention (B=2, S=2048, D=1024, H=16) on 8 Trainium2 cores.

Sharding: core c handles batch b = c//4 and head group g = c%4 (4 heads,
256 of the 1024 QKV output columns).

Design (v5, ACT/exp-bound target):
  - All matmuls bf16, fp32 PSUM. QKV projections stream xT windows of 512
    against W.T column blocks; q/k psums evict on DVE with bias folded in
    (tensor_scalar_add) into qT/kT [128p (2 heads x 64 hd), blk, S].
  - Attention per (hp, j, i) slot: the two heads' K=64 logits matmuls sit
    on disjoint PE row groups (partitions 0-63 / 64-127) and execute
    CONCURRENTLY (row tiling, ~360 ns for the pair); one [128,1024] exp on
    ACT covers both heads; DVE mask-multiply with a stride-0 broadcast of
    the keepT slice; PV accumulates with the ones-augmented V (row 64 =
    softmax denominator).
  - No PE output transposes: the block drain normalizes in the transposed
    layout (DVE reciprocal of the denominator row, PE ones-broadcast of
    the reciprocal into PSUM, DVE multiply) and DMAs oT [COLS, S]; the
    host does the final pure-layout transpose (same class of prep as the
    xT/keepT input layout work).
  - Startup: dummy warm-up matmuls keep the PE HAM clock at 2.4 GHz while
    the first weight/xT DMAs land; wq and keepT(j0) ride the ACT hwdge
    queue in parallel with wk/xT on the SP queue; projections beyond the
    first k/q window ride in PE slack during attention via a
    deadline-driven filler queue.
"""

import numpy as np

B, S, D, H = 2, 2048, 1024, 16
HD = D // H  # 64
HEADS_PER_CORE = 4
COLS = HEADS_PER_CORE * HD  # 256
N_CORES = 8
KT = D // 128  # 8 contraction tiles for projections
ST = S // 128  # 16 s tiles
NW = 4  # 512-wide windows
SCALE = 1.0 / np.sqrt(np.float32(D))

_cache = {}


def _build_nc():
    import concourse.bass as bass
    import concourse.mybir as mybir
    import concourse.tile as tile

    f32 = mybir.dt.float32
    bf16 = mybir.dt.bfloat16

    nc = bass.Bass(trn_type="TRN2")

    xT = nc.dram_tensor("xT", [D, S], bf16, kind="ExternalInput")
    wq = nc.dram_tensor("wq", [D, COLS], bf16, kind="ExternalInput")
    wk = nc.dram_tensor("wk", [D, COLS], bf16, kind="ExternalInput")
    wv = nc.dram_tensor("wv", [D, COLS], bf16, kind="ExternalInput")
    bq = nc.dram_tensor("bq", [128, 2], f32, kind="ExternalInput")
    bk = nc.dram_tensor("bk", [128, 2], f32, kind="ExternalInput")
    bv = nc.dram_tensor("bv", [1, COLS], bf16, kind="ExternalInput")
    keepT = nc.dram_tensor("keepT", [S, S], bf16, kind="ExternalInput")
    oT = nc.dram_tensor("oT", [COLS, S], f32, kind="ExternalOutput")

    with tile.TileContext(nc) as tc:
        with (
            tc.tile_pool(name="singles", bufs=1) as singles,
            tc.tile_pool(name="persist", bufs=1) as persist,
            tc.tile_pool(name="big_ps", bufs=2, space="PSUM") as big_ps,
            tc.tile_pool(name="pv_ps", bufs=2, space="PSUM") as pv_ps,
            tc.tile_pool(name="proj_ps", bufs=1, space="PSUM") as proj_ps,
            tc.tile_pool(name="bc_ps", bufs=1, space="PSUM") as bc_ps,
            tc.tile_pool(name="expw", bufs=4) as expw_pool,
            tc.tile_pool(name="expw2", bufs=4) as expw2_pool,
            tc.tile_pool(name="tails", bufs=4) as tails,
        ):
            # ---- constants ----
            ones_col = singles.tile([1, 128], bf16)
            nc.vector.memset(ones_col, 1.0)
            id_bf = singles.tile([128, 512], bf16)
            nc.vector.memset(id_bf, 1.0)
            bq_sb = singles.tile([128, 2], f32)
            bk_sb = singles.tile([128, 2], f32)
            bv_sb = singles.tile([1, COLS], bf16)

            # ---- persistent buffers ----
            wq_sb = persist.tile([128, KT, COLS], bf16)
            wk_sb = persist.tile([128, KT, COLS], bf16)
            wv_sb = persist.tile([128, KT, COLS], bf16)
            xT_sb = persist.tile([128, KT, S], bf16)
            keepT_sb = persist.tile([128, ST, S], bf16)
            qT_sb = persist.tile([128, 2, S], bf16)
            kT_sb = persist.tile([128, 2, S], bf16)
            v_aug = persist.tile([128, ST, HEADS_PER_CORE, HD + 1], bf16)
            nc.vector.memset(v_aug[:, :, :, HD : HD + 1], 1.0)

            # ---- DMA issue (order = priority) ----
            xT_r = xT[:, :].rearrange("(kt p) s -> p kt s", p=128)
            keepT_r = keepT[:, :].rearrange("(i p) s -> p i s", p=128)

            def dma_xT_w(w):
                nc.sync.dma_start(
                    out=xT_sb[:, :, w * 512 : (w + 1) * 512],
                    in_=xT_r[:, :, w * 512 : (w + 1) * 512],
                )

            def dma_keep_j(j, i0, i1, eng):
                eng.dma_start(
                    out=keepT_sb[:, i0:i1, j * 512 : (j + 1) * 512],
                    in_=keepT_r[:, i0:i1, j * 512 : (j + 1) * 512],
                )

            def dma_w(w_sb, w_dram, eng):
                eng.dma_start(
                    out=w_sb,
                    in_=w_dram[:, :].rearrange("(kt p) c -> p kt c", p=128),
                )

            # ACT queue: wq + first keep block (ACT is idle until first exp)
            dma_w(wq_sb, wq, nc.scalar)
            dma_keep_j(0, 0, 8, nc.scalar)
            dma_keep_j(0, 8, 16, nc.scalar)
            # SP queue: everything else, payload-first
            dma_w(wk_sb, wk, nc.sync)
            dma_xT_w(0)
            nc.sync.dma_start(out=bk_sb, in_=bk[:, :])
            nc.sync.dma_start(out=bq_sb, in_=bq[:, :])
            dma_xT_w(1)
            dma_w(wv_sb, wv, nc.sync)
            nc.sync.dma_start(out=bv_sb, in_=bv[:, :])
            dma_xT_w(2)
            dma_xT_w(3)
            dma_keep_j(1, 0, 16, nc.sync)
            dma_keep_j(2, 0, 16, nc.sync)
            dma_keep_j(3, 0, 16, nc.sync)

            # ---- PE warm-up: garbage matmuls on the identity tile keep the
            # HAM activity window busy while the first DMAs land, so the
            # first projection runs at 2.4 GHz instead of 1.2.
            warm = proj_ps.tile([128, 512], f32, tag="proj")
            for r in range(20):
                nc.tensor.matmul(
                    warm,
                    lhsT=id_bf[:, 0:128],
                    rhs=id_bf[:, :],
                    start=(r == 0),
                    stop=(r == 19),
                    skip_group_check=True,
                )

            # ---- projection groups ----
            def proj_qk(which, blk, w):
                w_sb, b_sb, dst = (
                    (wq_sb, bq_sb, qT_sb),
                    (wk_sb, bk_sb, kT_sb),
                )[which]
                ps = proj_ps.tile([128, 512], f32, tag="proj")
                for kt in range(KT):
                    nc.tensor.matmul(
                        ps,
                        lhsT=w_sb[:, kt, blk * 128 : (blk + 1) * 128],
                        rhs=xT_sb[:, kt, w * 512 : (w + 1) * 512],
                        start=(kt == 0),
                        stop=(kt == KT - 1),
                        skip_group_check=True,
                    )
                nc.vector.tensor_scalar_add(
                    out=dst[:, blk, w * 512 : (w + 1) * 512],
                    in0=ps,
                    scalar1=b_sb[:, blk : blk + 1],
                )

            def proj_v(st):
                psv = proj_ps.tile([128, COLS], f32, tag="proj")
                nc.tensor.matmul(
                    psv,
                    lhsT=ones_col[:, :],
                    rhs=bv_sb[:, :],
                    start=True,
                    stop=False,
                    skip_group_check=True,
                )
                for kt in range(KT):
                    nc.tensor.matmul(
                        psv,
                        lhsT=xT_sb[:, kt, st * 128 : (st + 1) * 128],
                        rhs=wv_sb[:, kt, :],
                        start=False,
                        stop=(kt == KT - 1),
                        skip_group_check=True,
                    )
                nc.vector.tensor_copy(
                    out=v_aug[:, st, :, 0:HD],
                    in_=psv.rearrange("p (h d) -> p h d", h=HEADS_PER_CORE),
                )

            # Filler queue: (deadline_slot, thunk). Slot = (hp*4 + j)*16 + i.
            # Fillers run after the slot's QK and before its PV. Deadlines
            # sit a few slots before first use so the DVE evictions never
            # land immediately ahead of a block seam.
            fillers = []
            for w in range(1, NW):
                fillers.append((max(0, 4 * w - 2), lambda w=w: proj_qk(1, 0, w)))
            for st in range(ST):
                fillers.append((st, lambda st=st: proj_v(st)))
            for w in range(1, NW):
                fillers.append((16 * w - 6, lambda w=w: proj_qk(0, 0, w)))
            for w in range(NW):
                fillers.append((28 + 4 * w, lambda w=w: proj_qk(1, 1, w)))
            for w in range(NW):
                fillers.append((44 + 4 * w, lambda w=w: proj_qk(0, 1, w)))
            fillers.sort(key=lambda t: t[0])

            def make_drain(hp, j, pvs):
                def drain():
                    for e in range(2):
                        h = 2 * hp + e
                        pv_sb = tails.tile(
                            [HD + 1, 512], f32, tag="pvsb", name=f"pv_sb{e}"
                        )
                        nc.vector.tensor_copy(out=pv_sb, in_=pvs[e])
                        # reciprocal of the denominator row, broadcast
                        # across the 64 head dims via a rank-1 matmul
                        rr = tails.tile([1, 512], bf16, tag="rr")
                        with nc.allow_low_precision(
                            reason="bf16 reciprocal row for rank-1 broadcast"
                        ):
                            nc.vector.reciprocal(
                                out=rr, in_=pv_sb[HD : HD + 1, :]
                            )
                        bc = bc_ps.tile([HD, 512], f32, tag="bc")
                        nc.tensor.matmul(
                            bc,
                            lhsT=ones_col[:, 0:HD],
                            rhs=rr,
                            start=True,
                            stop=True,
                            skip_group_check=True,
                        )
                        obT = tails.tile([HD, 512], f32, tag="obT")
                        nc.vector.tensor_mul(
                            out=obT, in0=pv_sb[0:HD, :], in1=bc
                        )
                        nc.sync.dma_start(
                            out=oT[h * HD : (h + 1) * HD, j * 512 : (j + 1) * 512],
                            in_=obT,
                        )

                return drain

            # Preamble: first k/q windows so attention starts immediately.
            proj_qk(1, 0, 0)  # k blk0 w0
            proj_qk(0, 0, 0)  # q blk0 w0

            pending_drain = None
            pvs = None
            for slot in range(2 * NW * ST):
                hp, rem = divmod(slot, NW * ST)
                j, i = divmod(rem, ST)
                # QK + exp + mask first: keeps ACT fed across block seams.
                lgp = big_ps.tile([128, 1024], f32, tag="big")
                for e in range(2):
                    po = e * 64
                    nc.tensor.matmul(
                        lgp[:, e * 512 : (e + 1) * 512],
                        lhsT=kT_sb[po : po + 64, hp, i * 128 : (i + 1) * 128],
                        rhs=qT_sb[po : po + 64, hp, j * 512 : (j + 1) * 512],
                        start=True,
                        stop=True,
                        skip_group_check=True,
                    )
                ex = expw_pool.tile([128, 1024], bf16)
                nc.scalar.activation(
                    out=ex,
                    in_=lgp,
                    func=mybir.ActivationFunctionType.Exp,
                    scale=float(SCALE),
                )
                ex2 = expw2_pool.tile([128, 1024], bf16)
                k_ap = keepT_sb[:, i, j * 512 : (j + 1) * 512]
                k_bcast = bass.AP(
                    tensor=k_ap.tensor,
                    offset=k_ap.offset,
                    ap=[k_ap.ap[0], [0, 2], *k_ap.ap[1:]],
                )
                nc.vector.tensor_mul(
                    out=ex2.rearrange("p (e n) -> p e n", e=2),
                    in0=ex.rearrange("p (e n) -> p e n", e=2),
                    in1=k_bcast,
                )
                if i == 0:
                    if pending_drain is not None:
                        pending_drain()
                        pending_drain = None
                while fillers and fillers[0][0] <= slot:
                    fillers.pop(0)[1]()
                if i == 0:
                    pvs = [
                        pv_ps.tile([HD + 1, 512], f32, tag="pv", name=f"pv{e}")
                        for e in range(2)
                    ]
                for e in range(2):
                    nc.tensor.matmul(
                        pvs[e],
                        lhsT=v_aug[:, i, 2 * hp + e, :],
                        rhs=ex2[:, e * 512 : (e + 1) * 512],
                        start=(i == 0),
                        stop=(i == ST - 1),
                        skip_group_check=True,
                    )
                if i == ST - 1:
                    pending_drain = make_drain(hp, j, pvs)
            pending_drain()

    _split_multiwait(nc)
    return nc


def _split_multiwait(nc, max_waits: int = 1):
    import concourse.mybir as mybir

    for f in nc.m.functions:
        for blk in f.blocks:
            out = []
            changed = False
            for inst in blk.instructions:
                si = inst.sync_info
                if si is not None and len(si.on_wait) > max_waits:
                    waits = list(si.on_wait)
                    extra = waits[: len(waits) - max_waits]
                    keep = waits[len(waits) - max_waits :]
                    for k, w in enumerate(extra):
                        out.append(
                            mybir.InstNoOp(
                                name=f"{inst.name}-wfx{k}",
                                engine=inst.engine,
                                sync_info=mybir.SyncInfo(on_wait=[w], on_update=[]),
                                bass_nofuse=True,
                            )
                        )
                    inst.sync_info = mybir.SyncInfo(
                        on_wait=keep, on_update=list(si.on_update)
                    )
                    changed = True
                out.append(inst)
            if changed:
                blk.instructions = out
    return nc


def _prep_in_maps(x, mask, Wq, bq, Wk, bk, Wv, bv):
    import ml_dtypes

    bf16 = ml_dtypes.bfloat16
    x = np.asarray(x, np.float32)
    mask = np.asarray(mask, bool)

    xT_b = [np.ascontiguousarray(x[b].T).astype(bf16) for b in range(B)]
    keepT_b = [
        np.ascontiguousarray((~mask[b, 0]).T).astype(bf16) for b in range(B)
    ]
    WqT = np.asarray(Wq, np.float32).T.astype(bf16)
    WkT = np.asarray(Wk, np.float32).T.astype(bf16)
    WvT = np.asarray(Wv, np.float32).T.astype(bf16)
    bq32 = np.asarray(bq, np.float32)
    bk32 = np.asarray(bk, np.float32)
    bv = np.asarray(bv, np.float32).astype(bf16)

    in_maps = []
    for c in range(N_CORES):
        b, g = divmod(c, 4)
        cols = slice(g * COLS, (g + 1) * COLS)
        in_maps.append(
            {
                "xT": xT_b[b],
                "wq": np.ascontiguousarray(WqT[:, cols]),
                "wk": np.ascontiguousarray(WkT[:, cols]),
                "wv": np.ascontiguousarray(WvT[:, cols]),
                "bq": np.ascontiguousarray(bq32[cols].reshape(2, 128).T),
                "bk": np.ascontiguousarray(bk32[cols].reshape(2, 128).T),
                "bv": np.ascontiguousarray(bv[cols].reshape(1, COLS)),
                "keepT": keepT_b[b],
            }
        )
    return in_maps


def kernel(x, mask, Wq, bq, Wk, bk, Wv, bv, _trace=False):
    from concourse.bass_utils import run_bass_kernel_spmd

    if "nc" not in _cache:
        _cache["nc"] = _build_nc()
    nc = _cache["nc"]

    in_maps = _prep_in_maps(x, mask, Wq, bq, Wk, bk, Wv, bv)
    res = run_bass_kernel_spmd(
        nc, in_maps, core_ids=list(range(N_CORES)), trace=_trace
    )
    _cache["last_result"] = res

    out = np.empty((B, S, D), np.float32)
    for c in range(N_CORES):
        b, g = divmod(c, 4)
        out[b, :, g * COLS : (g + 1) * COLS] = res.results[c]["oT"].T
    return out


# revision 27
# speedup vs baseline: 1.3930x; 1.3930x over previous
"""Multi-head attention (B=2, S=2048, D=1024, H=16) on 8 Trainium2 cores.

Sharding: core c handles batch b = c//4 and head group g = c%4 (4 heads,
256 of the 1024 QKV output columns).

Design (v5, ACT/exp-bound target):
  - All matmuls bf16, fp32 PSUM. QKV projections stream xT windows of 512
    against W.T column blocks; q/k psums evict on DVE with bias folded in
    (tensor_scalar_add) into qT/kT [128p (2 heads x 64 hd), blk, S].
  - Attention per (hp, j, i) slot: the two heads' K=64 logits matmuls sit
    on disjoint PE row groups (partitions 0-63 / 64-127) and execute
    CONCURRENTLY (row tiling, ~360 ns for the pair); one [128,1024] exp on
    ACT covers both heads; DVE mask-multiply with a stride-0 broadcast of
    the keepT slice; PV accumulates with the ones-augmented V (row 64 =
    softmax denominator).
  - Block drains are split into 3 thunks consumed one per slot (pv
    eviction, then per-head: 4 PE transposes into one [128,4,65] psum
    tile, a 4-wide reciprocal of the denominators, one broadcast
    multiply, output DMA) so drain work never sits ahead of a seam's QK
    on the PE queue.
  - Startup: dummy warm-up matmuls keep the PE HAM clock at 2.4 GHz while
    the first weight/xT DMAs land; wq and keepT(j0) ride the ACT hwdge
    queue in parallel with wk/xT on the SP queue; projections beyond the
    first k/q window ride in PE slack during attention via a
    deadline-driven filler queue.
"""

import numpy as np

B, S, D, H = 2, 2048, 1024, 16
HD = D // H  # 64
HEADS_PER_CORE = 4
COLS = HEADS_PER_CORE * HD  # 256
N_CORES = 8
KT = D // 128  # 8 contraction tiles for projections
ST = S // 128  # 16 s tiles
NW = 4  # 512-wide windows
SCALE = 1.0 / np.sqrt(np.float32(D))

_cache = {}


def _build_nc():
    import concourse.bass as bass
    import concourse.mybir as mybir
    import concourse.tile as tile
    from concourse.masks import make_identity

    f32 = mybir.dt.float32
    bf16 = mybir.dt.bfloat16

    nc = bass.Bass(trn_type="TRN2")

    xT = nc.dram_tensor("xT", [D, S], bf16, kind="ExternalInput")
    wq = nc.dram_tensor("wq", [D, COLS], bf16, kind="ExternalInput")
    wk = nc.dram_tensor("wk", [D, COLS], bf16, kind="ExternalInput")
    wv = nc.dram_tensor("wv", [D, COLS], bf16, kind="ExternalInput")
    bq = nc.dram_tensor("bq", [128, 2], f32, kind="ExternalInput")
    bk = nc.dram_tensor("bk", [128, 2], f32, kind="ExternalInput")
    bv = nc.dram_tensor("bv", [1, COLS], bf16, kind="ExternalInput")
    keepT = nc.dram_tensor("keepT", [S, S], bf16, kind="ExternalInput")
    o = nc.dram_tensor("o", [S, COLS], f32, kind="ExternalOutput")

    with tile.TileContext(nc) as tc:
        with (
            tc.tile_pool(name="singles", bufs=1) as singles,
            tc.tile_pool(name="persist", bufs=1) as persist,
            tc.tile_pool(name="big_ps", bufs=2, space="PSUM") as big_ps,
            tc.tile_pool(name="pv_ps", bufs=2, space="PSUM") as pv_ps,
            tc.tile_pool(name="proj_ps", bufs=1, space="PSUM") as proj_ps,
            tc.tile_pool(name="tr_ps", bufs=1, space="PSUM") as tr_ps,
            tc.tile_pool(name="expw", bufs=4) as expw_pool,
            tc.tile_pool(name="expw2", bufs=4) as expw2_pool,
            tc.tile_pool(name="tails", bufs=4) as tails,
        ):
            # ---- constants ----
            ones_col = singles.tile([1, 128], bf16)
            nc.vector.memset(ones_col, 1.0)
            identity = singles.tile([128, 128], f32)
            make_identity(nc, identity)
            id_bf = singles.tile([128, 512], bf16)
            nc.vector.memset(id_bf, 1.0)
            bq_sb = singles.tile([128, 2], f32)
            bk_sb = singles.tile([128, 2], f32)
            bv_sb = singles.tile([1, COLS], bf16)

            # ---- persistent buffers ----
            wq_sb = persist.tile([128, KT, COLS], bf16)
            wk_sb = persist.tile([128, KT, COLS], bf16)
            wv_sb = persist.tile([128, KT, COLS], bf16)
            xT_sb = persist.tile([128, KT, S], bf16)
            keepT_sb = persist.tile([128, ST, S], bf16)
            qT_sb = persist.tile([128, 2, S], bf16)
            kT_sb = persist.tile([128, 2, S], bf16)
            v_aug = persist.tile([128, ST, HEADS_PER_CORE, HD + 1], bf16)
            nc.vector.memset(v_aug[:, :, :, HD : HD + 1], 1.0)

            # ---- DMA issue (order = priority) ----
            xT_r = xT[:, :].rearrange("(kt p) s -> p kt s", p=128)
            keepT_r = keepT[:, :].rearrange("(i p) s -> p i s", p=128)

            def dma_xT_w(w):
                nc.sync.dma_start(
                    out=xT_sb[:, :, w * 512 : (w + 1) * 512],
                    in_=xT_r[:, :, w * 512 : (w + 1) * 512],
                )

            def dma_keep_j(j, i0, i1, eng):
                eng.dma_start(
                    out=keepT_sb[:, i0:i1, j * 512 : (j + 1) * 512],
                    in_=keepT_r[:, i0:i1, j * 512 : (j + 1) * 512],
                )

            def dma_w(w_sb, w_dram, eng):
                eng.dma_start(
                    out=w_sb,
                    in_=w_dram[:, :].rearrange("(kt p) c -> p kt c", p=128),
                )

            # ACT queue: wq + first keep block (ACT is idle until first exp)
            dma_w(wq_sb, wq, nc.scalar)
            dma_keep_j(0, 0, 8, nc.scalar)
            dma_keep_j(0, 8, 16, nc.scalar)
            # SP queue: everything else, payload-first
            dma_w(wk_sb, wk, nc.sync)
            dma_xT_w(0)
            nc.sync.dma_start(out=bk_sb, in_=bk[:, :])
            nc.sync.dma_start(out=bq_sb, in_=bq[:, :])
            dma_xT_w(1)
            dma_w(wv_sb, wv, nc.sync)
            nc.sync.dma_start(out=bv_sb, in_=bv[:, :])
            dma_xT_w(2)
            dma_xT_w(3)
            dma_keep_j(1, 0, 16, nc.sync)
            dma_keep_j(2, 0, 16, nc.sync)
            dma_keep_j(3, 0, 16, nc.sync)

            # ---- PE warm-up: garbage matmuls on the identity tile keep the
            # HAM activity window busy while the first DMAs land, so the
            # first projection runs at 2.4 GHz instead of 1.2.
            warm = proj_ps.tile([128, 512], f32, tag="proj")
            for r in range(20):
                nc.tensor.matmul(
                    warm,
                    lhsT=id_bf[:, 0:128],
                    rhs=id_bf[:, :],
                    start=(r == 0),
                    stop=(r == 19),
                    skip_group_check=True,
                )

            # ---- projection groups ----
            def proj_qk(which, blk, w):
                w_sb, b_sb, dst = (
                    (wq_sb, bq_sb, qT_sb),
                    (wk_sb, bk_sb, kT_sb),
                )[which]
                ps = proj_ps.tile([128, 512], f32, tag="proj")
                for kt in range(KT):
                    nc.tensor.matmul(
                        ps,
                        lhsT=w_sb[:, kt, blk * 128 : (blk + 1) * 128],
                        rhs=xT_sb[:, kt, w * 512 : (w + 1) * 512],
                        start=(kt == 0),
                        stop=(kt == KT - 1),
                        skip_group_check=True,
                    )
                nc.vector.tensor_scalar_add(
                    out=dst[:, blk, w * 512 : (w + 1) * 512],
                    in0=ps,
                    scalar1=b_sb[:, blk : blk + 1],
                )

            def proj_v(st):
                psv = proj_ps.tile([128, COLS], f32, tag="proj")
                nc.tensor.matmul(
                    psv,
                    lhsT=ones_col[:, :],
                    rhs=bv_sb[:, :],
                    start=True,
                    stop=False,
                    skip_group_check=True,
                )
                for kt in range(KT):
                    nc.tensor.matmul(
                        psv,
                        lhsT=xT_sb[:, kt, st * 128 : (st + 1) * 128],
                        rhs=wv_sb[:, kt, :],
                        start=False,
                        stop=(kt == KT - 1),
                        skip_group_check=True,
                    )
                nc.vector.tensor_copy(
                    out=v_aug[:, st, :, 0:HD],
                    in_=psv.rearrange("p (h d) -> p h d", h=HEADS_PER_CORE),
                )

            # Filler queue: (deadline_slot, thunk). Slot = (hp*4 + j)*16 + i.
            # Fillers run after the slot's QK and before its PV. Deadlines
            # sit a few slots before first use so the DVE evictions never
            # land immediately ahead of a block seam.
            fillers = []
            for w in range(1, NW):
                fillers.append((max(0, 4 * w - 2), lambda w=w: proj_qk(1, 0, w)))
            for st in range(ST):
                fillers.append((st, lambda st=st: proj_v(st)))
            for w in range(1, NW):
                fillers.append((16 * w - 6, lambda w=w: proj_qk(0, 0, w)))
            for w in range(NW):
                fillers.append((28 + 4 * w, lambda w=w: proj_qk(1, 1, w)))
            for w in range(NW):
                fillers.append((44 + 4 * w, lambda w=w: proj_qk(0, 1, w)))
            fillers.sort(key=lambda t: t[0])

            def drain_thunks(hp, j, pvs):
                """Split the block drain into 3 thunks consumed one per
                slot, so drain work never sits ahead of a seam's QK on the
                PE queue."""
                pv_sbs = []

                def evict():
                    for e in range(2):
                        pv_sb = tails.tile(
                            [HD + 1, 512], f32, tag="pvsb", name=f"pv_sb{e}"
                        )
                        nc.vector.tensor_copy(out=pv_sb, in_=pvs[e])
                        pv_sbs.append(pv_sb)

                def norm(e):
                    def go():
                        h = 2 * hp + e
                        pv_sb = pv_sbs[e]
                        ob = tails.tile([128, 4, HD], f32, tag="ob")
                        tr = tr_ps.tile([128, 4, HD + 1], f32, tag="tr")
                        for c in range(4):
                            nc.tensor.transpose(
                                out=tr[:, c, :],
                                in_=pv_sb[:, c * 128 : (c + 1) * 128],
                                identity=identity[0 : HD + 1, 0 : HD + 1],
                            )
                        rc = tails.tile([128, 4], f32, tag="rc")
                        nc.vector.reciprocal(out=rc, in_=tr[:, :, HD : HD + 1])
                        rc_ap = rc[:, :]
                        rc_bcast = bass.AP(
                            tensor=rc_ap.tensor,
                            offset=rc_ap.offset,
                            ap=[*rc_ap.ap, [0, HD]],
                        )
                        nc.vector.tensor_mul(
                            out=ob, in0=tr[:, :, 0:HD], in1=rc_bcast
                        )
                        nc.sync.dma_start(
                            out=o[
                                j * 512 : (j + 1) * 512, h * HD : (h + 1) * HD
                            ].rearrange("(c p) d -> p c d", p=128),
                            in_=ob,
                        )

                    return go

                return [evict, norm(0), norm(1)]

            # Preamble: first k/q windows so attention starts immediately.
            proj_qk(1, 0, 0)  # k blk0 w0
            proj_qk(0, 0, 0)  # q blk0 w0

            drainq = []
            pvs = None
            for slot in range(2 * NW * ST):
                hp, rem = divmod(slot, NW * ST)
                j, i = divmod(rem, ST)
                # QK + exp + mask first: keeps ACT fed across block seams.
                lgp = big_ps.tile([128, 1024], f32, tag="big")
                for e in range(2):
                    po = e * 64
                    nc.tensor.matmul(
                        lgp[:, e * 512 : (e + 1) * 512],
                        lhsT=kT_sb[po : po + 64, hp, i * 128 : (i + 1) * 128],
                        rhs=qT_sb[po : po + 64, hp, j * 512 : (j + 1) * 512],
                        start=True,
                        stop=True,
                        skip_group_check=True,
                    )
                ex = expw_pool.tile([128, 1024], bf16)
                nc.scalar.activation(
                    out=ex,
                    in_=lgp,
                    func=mybir.ActivationFunctionType.Exp,
                    scale=float(SCALE),
                )
                ex2 = expw2_pool.tile([128, 1024], bf16)
                k_ap = keepT_sb[:, i, j * 512 : (j + 1) * 512]
                k_bcast = bass.AP(
                    tensor=k_ap.tensor,
                    offset=k_ap.offset,
                    ap=[k_ap.ap[0], [0, 2], *k_ap.ap[1:]],
                )
                nc.vector.tensor_mul(
                    out=ex2.rearrange("p (e n) -> p e n", e=2),
                    in0=ex.rearrange("p (e n) -> p e n", e=2),
                    in1=k_bcast,
                )
                if drainq:
                    drainq.pop(0)()
                while fillers and fillers[0][0] <= slot:
                    fillers.pop(0)[1]()
                if i == 0:
                    pvs = [
                        pv_ps.tile([HD + 1, 512], f32, tag="pv", name=f"pv{e}")
                        for e in range(2)
                    ]
                for e in range(2):
                    nc.tensor.matmul(
                        pvs[e],
                        lhsT=v_aug[:, i, 2 * hp + e, :],
                        rhs=ex2[:, e * 512 : (e + 1) * 512],
                        start=(i == 0),
                        stop=(i == ST - 1),
                        skip_group_check=True,
                    )
                if i == ST - 1:
                    drainq.extend(drain_thunks(hp, j, pvs))
            while drainq:
                drainq.pop(0)()

    _split_multiwait(nc)
    return nc


def _split_multiwait(nc, max_waits: int = 1):
    import concourse.mybir as mybir

    for f in nc.m.functions:
        for blk in f.blocks:
            out = []
            changed = False
            for inst in blk.instructions:
                si = inst.sync_info
                if si is not None and len(si.on_wait) > max_waits:
                    waits = list(si.on_wait)
                    extra = waits[: len(waits) - max_waits]
                    keep = waits[len(waits) - max_waits :]
                    for k, w in enumerate(extra):
                        out.append(
                            mybir.InstNoOp(
                                name=f"{inst.name}-wfx{k}",
                                engine=inst.engine,
                                sync_info=mybir.SyncInfo(on_wait=[w], on_update=[]),
                                bass_nofuse=True,
                            )
                        )
                    inst.sync_info = mybir.SyncInfo(
                        on_wait=keep, on_update=list(si.on_update)
                    )
                    changed = True
                out.append(inst)
            if changed:
                blk.instructions = out
    return nc


def _prep_in_maps(x, mask, Wq, bq, Wk, bk, Wv, bv):
    import ml_dtypes

    bf16 = ml_dtypes.bfloat16
    x = np.asarray(x, np.float32)
    mask = np.asarray(mask, bool)

    xT_b = [np.ascontiguousarray(x[b].T).astype(bf16) for b in range(B)]
    keepT_b = [
        np.ascontiguousarray((~mask[b, 0]).T).astype(bf16) for b in range(B)
    ]
    WqT = np.asarray(Wq, np.float32).T.astype(bf16)
    WkT = np.asarray(Wk, np.float32).T.astype(bf16)
    WvT = np.asarray(Wv, np.float32).T.astype(bf16)
    bq32 = np.asarray(bq, np.float32)
    bk32 = np.asarray(bk, np.float32)
    bv = np.asarray(bv, np.float32).astype(bf16)

    in_maps = []
    for c in range(N_CORES):
        b, g = divmod(c, 4)
        cols = slice(g * COLS, (g + 1) * COLS)
        in_maps.append(
            {
                "xT": xT_b[b],
                "wq": np.ascontiguousarray(WqT[:, cols]),
                "wk": np.ascontiguousarray(WkT[:, cols]),
                "wv": np.ascontiguousarray(WvT[:, cols]),
                "bq": np.ascontiguousarray(bq32[cols].reshape(2, 128).T),
                "bk": np.ascontiguousarray(bk32[cols].reshape(2, 128).T),
                "bv": np.ascontiguousarray(bv[cols].reshape(1, COLS)),
                "keepT": keepT_b[b],
            }
        )
    return in_maps


def kernel(x, mask, Wq, bq, Wk, bk, Wv, bv, _trace=False):
    from concourse.bass_utils import run_bass_kernel_spmd

    if "nc" not in _cache:
        _cache["nc"] = _build_nc()
    nc = _cache["nc"]

    in_maps = _prep_in_maps(x, mask, Wq, bq, Wk, bk, Wv, bv)
    res = run_bass_kernel_spmd(
        nc, in_maps, core_ids=list(range(N_CORES)), trace=_trace
    )
    _cache["last_result"] = res

    out = np.empty((B, S, D), np.float32)
    for c in range(N_CORES):
        b, g = divmod(c, 4)
        out[b, :, g * COLS : (g + 1) * COLS] = res.results[c]["o"]
    return out
